# revision 1
# baseline (speedup 1.0000x reference)
"""Trainium2 Bass kernel for the Anisotropic Sliced-Wasserstein encoder
(segment_reduce): project [N,512] node features through [128,64] projections
(4 WL slices), sort each of the 256 projected columns within each of 1000
graph segments, and extract 100 quantiles per segment.

Strategy (8 NeuronCores, pure data-parallel over graphs, no collectives):
  host: split graphs 125/core; pad each segment to L (=max count, even) with a
        synthetic node row that projects to +1e4 for every projection column
        (pads sort to the top and never collide with quantile ranks);
        pack columns element-major (col = elem*S + seg) and pre-transpose so
        the device sees xt [512, L*S] bf16 per core.
  dev:  DMA xt tiles -> PE matmul with the (scale-folded) projections ->
        evict PSUM to two bf16 sort buffers [128 rows = (slice,proj), L*S] ->
        bitonic sort network (all-ascending reversal formulation, pruned to
        length L, ping-pong between buffers, every op 2x-mode eligible:
        seg dim innermost contiguous) -> DMA sorted buffers out.
  host: gather quantiles (ranks are host-known from `batch`) and assemble
        the [1000, 25600] float32 output.
"""
import numpy as np
import ml_dtypes

BF = ml_dtypes.bfloat16
NCORES = 8
G = 1000
POW = 2.0
BIG = 1e4


# ---------------------------------------------------------------------------
# Bitonic network descriptors (validated against np.sort in development).
# ---------------------------------------------------------------------------
def gen_rounds(L, n=256):
    assert L % 2 == 0 and L <= n
    rounds = []
    m = 1
    while m < n:
        ops = []
        bs = 2 * m
        nb_full = L // bs
        if nb_full:
            ops.append(("cmpx", 0, 2 * m - 1, bs, nb_full, m, -1))
        b0 = nb_full * bs
        if b0 < L:
            i0 = max(0, b0 + 2 * m - L)
            if i0 < m and b0 + m < L:
                run = m - i0
                ops.append(("cmpx", b0 + i0, b0 + 2 * m - 1 - i0, 0, 1, run, -1))
                if i0 > 0:
                    ops.append(("copy", b0, 0, 1, i0))
            else:
                ops.append(("copy", b0, 0, 1, L - b0))
        rounds.append(ops)
        d = m // 2
        while d >= 1:
            ops = []
            bs = 2 * d
            nb_full = L // bs
            if nb_full:
                ops.append(("cmpx", 0, d, bs, nb_full, d, +1))
            b0 = nb_full * bs
            if b0 < L:
                run_p = max(0, L - b0 - d)
                if run_p:
                    ops.append(("cmpx", b0, b0 + d, 0, 1, run_p, +1))
                cs = b0 + run_p
                ce = min(b0 + d, L)
                if ce > cs:
                    ops.append(("copy", cs, 0, 1, ce - cs))
            rounds.append(ops)
            d //= 2
        m *= 2
    return rounds


# ---------------------------------------------------------------------------
# Device kernel
# ---------------------------------------------------------------------------
_NC_CACHE = {}


def _eview(bass_mod, buf_ap, off, bs, nb, run, rstep, S):
    """View of buf [128, L*S] at elem positions off + b*bs + r*rstep,
    seg dim (S, stride 1) innermost."""
    part = list(buf_ap.ap[0])
    dims = [part]
    if nb > 1:
        dims.append([bs * S, nb])
    dims.append([rstep * S, run])
    dims.append([1, S])
    return bass_mod.AP(buf_ap.tensor, buf_ap.offset + off * S, dims)


def build_nc(L, S):
    key = (L, S)
    if key in _NC_CACHE:
        return _NC_CACHE[key]
    import concourse.bass as bass
    import concourse.bacc as bacc
    import concourse.mybir as mybir
    from concourse.tile import TileContext

    NCOL = L * S
    rounds = gen_rounds(L)
    bf = mybir.dt.bfloat16

    nc = bacc.Bacc("TRN2", target_bir_lowering=False, debug=False,
                   num_devices=NCORES)
    xt = nc.declare_dram_parameter("xt", [512, NCOL], bf, isOutput=False)
    proj = nc.declare_dram_parameter("proj", [128, 64], bf, isOutput=False)
    out = nc.declare_dram_parameter("sorted", [256, NCOL], bf, isOutput=True)

    MM = 500          # matmul free-dim chunk (psum bank limit 512 fp32)
    CH = 3500         # dma chunk (multiple of MM)

    with TileContext(nc) as tc:
        with (
            tc.tile_pool(name="const", bufs=1) as constp,
            tc.tile_pool(name="stage", bufs=3) as stagep,
            tc.tile_pool(name="psum", bufs=8, space="PSUM") as psump,
            tc.tile_pool(name="bufs", bufs=1) as bufp,
        ):
            projt = constp.tile([128, 64], bf)
            nc.sync.dma_start(projt[:], proj[:])

            bufA = bufp.tile([128, NCOL], bf, name="bufA", tag="bufA")
            bufB = bufp.tile([128, NCOL], bf, name="bufB", tag="bufB")
            bufZ = bufp.tile([128, NCOL], bf, name="bufZ", tag="bufZ")

            def fill(b, tgt):
                for ih in (0, 1):
                    i = 2 * b + ih
                    c0 = 0
                    while c0 < NCOL:
                        cw = min(CH, NCOL - c0)
                        st = stagep.tile([128, CH], bf, name="st", tag="st")
                        nc.sync.dma_start(st[:, :cw],
                                          xt[i * 128:(i + 1) * 128, c0:c0 + cw])
                        j0 = 0
                        while j0 < cw:
                            jw = min(MM, cw - j0)
                            ps = psump.tile([64, MM], mybir.dt.float32,
                                            name="ps", tag="ps")
                            nc.tensor.matmul(ps[:, :jw], lhsT=projt[:],
                                             rhs=st[:, j0:j0 + jw],
                                             start=True, stop=True)
                            nc.scalar.copy(
                                tgt[64 * ih:64 * ih + 64,
                                    c0 + j0:c0 + j0 + jw],
                                ps[:, :jw])
                            j0 += jw
                        c0 += cw

            def emit_sort(A, Z):
                cur, pong = A, Z
                for ops in rounds:
                    ca, pa = cur[:], pong[:]
                    for op in ops:
                        if op[0] == "cmpx":
                            _, lo, hi, bs, nb, run, hstep = op
                            slo = _eview(bass, ca, lo, bs, nb, run, +1, S)
                            shi = _eview(bass, ca, hi, bs, nb, run, hstep, S)
                            dlo = _eview(bass, pa, lo, bs, nb, run, +1, S)
                            dhi = _eview(bass, pa, hi, bs, nb, run, hstep, S)
                            nc.vector.tensor_tensor(dlo, slo, shi,
                                                    op=mybir.AluOpType.min)
                            nc.vector.tensor_tensor(dhi, slo, shi,
                                                    op=mybir.AluOpType.max)
                        else:
                            _, off, bs, nb, run = op
                            src = _eview(bass, ca, off, bs, nb, run, +1, S)
                            dst = _eview(bass, pa, off, bs, nb, run, +1, S)
                            nc.vector.tensor_copy(dst, src)
                    cur, pong = pong, cur
                assert cur is A  # even number of rounds
                return A

            fill(0, bufA)
            fill(1, bufB)
            emit_sort(bufA, bufZ)
            nc.sync.dma_start(out[0:128, :], bufA[:])
            emit_sort(bufB, bufZ)
            nc.sync.dma_start(out[128:256, :], bufB[:])

    nc.finalize()
    _NC_CACHE[key] = nc
    return nc


# ---------------------------------------------------------------------------
# Host side
# ---------------------------------------------------------------------------
def _host_prepare(x, batch, projections, cum_weights):
    N, DT = x.shape
    D, P = projections.shape
    I1 = DT // D
    Q = cum_weights.shape[0]
    counts = np.bincount(batch, minlength=G).astype(np.int64)
    starts = np.concatenate([[0], np.cumsum(counts)[:-1]]).astype(np.int64)
    L = int(max(counts.max(), 2))
    L += L % 2
    spc = G // NCORES
    S = spc + (spc % 2)
    qidx = np.floor(cum_weights[None, :].astype(np.float32)
                    * np.maximum(counts - 1, 0)[:, None].astype(np.float32)
                    ).astype(np.int64)
    scale = float((Q * P) ** (1.0 / POW))
    proj_s = np.ascontiguousarray(projections.astype(np.float32) / scale).astype(BF)
    proj_pad = np.zeros((128, 64), BF)
    proj_pad[:D, :P] = proj_s

    pf = projections.astype(np.float64)
    u_slice = pf @ np.linalg.solve(pf.T @ pf, np.full(P, BIG))
    u_row = np.tile(u_slice, I1).astype(np.float32)

    in_maps = []
    for c in range(NCORES):
        g0 = c * spc
        cnt = np.zeros(S, np.int64)
        cnt[:spc] = counts[g0:g0 + spc]
        st = np.zeros(S, np.int64)
        st[:spc] = starts[g0:g0 + spc]
        e = np.arange(L)[:, None]
        valid = e < cnt[None, :]
        idx = np.where(valid, st[None, :] + e, 0)
        cols = np.where(valid.reshape(-1, 1), x[idx.reshape(-1)],
                        u_row[None, :])                     # [L*S, 512]
        xt = np.ascontiguousarray(cols.T.astype(BF))        # [512, L*S]
        in_maps.append({"xt": xt, "proj": proj_pad})
    return in_maps, dict(L=L, S=S, spc=spc, qidx=qidx, Q=Q, P=P, I1=I1)


def _host_gather(sorted_list, meta):
    L, S, spc, Q, P, I1 = (meta["L"], meta["S"], meta["spc"], meta["Q"],
                           meta["P"], meta["I1"])
    qidx = meta["qidx"]
    out = np.empty((G, I1 * Q * P), np.float32)
    for c, srt in enumerate(sorted_list):
        a = np.asarray(srt).astype(np.float32).reshape(2, 2, 64, L, S)
        a = a.transpose(0, 1, 2, 4, 3)[:, :, :, :spc, :]    # [2,2,64,spc,L]
        qs = qidx[c * spc:(c + 1) * spc]                    # [spc, Q]
        sel = np.take_along_axis(a, qs[None, None, None, :, :], axis=4)
        out[c * spc:(c + 1) * spc] = sel.transpose(3, 0, 1, 4, 2).reshape(
            spc, I1 * Q * P)
    return out


def _run_device(in_maps, L, S, trace=False):
    from concourse.bass_utils import run_bass_kernel_spmd
    nc = build_nc(L, S)
    res = run_bass_kernel_spmd(nc, in_maps, core_ids=list(range(NCORES)),
                               trace=trace)
    return res


def kernel(x, batch, projections, cum_weights):
    x = np.asarray(x, dtype=np.float32)
    batch = np.asarray(batch)
    projections = np.asarray(projections, dtype=np.float32)
    cum_weights = np.asarray(cum_weights, dtype=np.float32)
    in_maps, meta = _host_prepare(x, batch, projections, cum_weights)
    res = _run_device(in_maps, meta["L"], meta["S"], trace=False)
    sorted_list = [res.results[c]["sorted"] for c in range(NCORES)]
    return _host_gather(sorted_list, meta)
